# revision 42
# baseline (speedup 1.0000x reference)
"""Trainium2 Bass kernel for nn_AtomLayer (threshold -> per-timestep argmax
over atoms -> per-(batch,atom) top-16 over time -> one-hot scatter).

Self-contained: hardcodes shapes B=64, A=256, T=2048, K=16, thr input.
Shards batch across 8 NeuronCores (8 batch elements per core).

Per core, per batch element b, software-pipelined two iterations ahead
(stage1 = load + valid + column-max; stage2 = mask + top-k + outputs):
  1. load x[b] as two [128, 2048] tiles (atoms on partitions)
  2. valid = Relu(Sign(x - thr)) on the Scalar engine (uint8) -> out
  3. column max over atoms: elementwise max of the two tiles (DVE) -> PE
     transpose (PSUM chunks) -> free-dim max-reduce -> m[t]; clamp
     m' = max(m, thr); flatten to a [1, T] row via PE transpose + SBUF DMA
  4. broadcast m' along atoms via K=1 ones-matmul into PSUM;
     y = x * (x >= m')  (eq on DVE from PSUM, one mult on Pool; keeps only
     column-max atoms passing the threshold, exact zeros elsewhere)
  5. top-16 per row with exact jax.lax.top_k tie-break semantics using the
     DVE max8 / max_index / match_replace instructions (first-occurrence
     matching reproduces "ties -> ascending index" exactly, including the
     zero-fill behaviour)
  6. indices out (int32); feat one-hot built in SBUF via gpsimd local_scatter
     (bf16 ones, per-partition int16 indices split into two 1024-wide
     halves; dst zeroed by the instruction) and DMA'd out as 2-byte payload
     (host converts bf16 -> f32, halving feat DRAM write traffic)

Engine balance per b (cost model): DVE ~36us (10 top-k passes = the floor),
Pool ~14us, ACT ~9us, PE ~6us, DMA ~28 MiB; ~301us/core total.
"""

import numpy as np

B, A, T = 64, 256, 2048
K = 16
NCORES = 8
B_SH = B // NCORES  # 8 batch elements per core

_CACHE = {}


def _build(b_sh=B_SH):
    import concourse.tile as tile
    from concourse import bacc, mybir
    from concourse.masks import make_identity

    P = 128
    HT = T // 2  # 1024, local_scatter num_elems limit is < 2048

    nc = bacc.Bacc("TRN2", target_bir_lowering=False, debug=False,
                   num_devices=NCORES)

    x_d = nc.dram_tensor("x", [b_sh, A, T], mybir.dt.float32,
                         kind="ExternalInput").ap()
    thr_d = nc.dram_tensor("threshold", [1, 1], mybir.dt.float32,
                           kind="ExternalInput").ap()
    feat_d = nc.dram_tensor("feat", [b_sh, A, T], mybir.dt.uint16,
                            kind="ExternalOutput").ap()
    idx_d = nc.dram_tensor("indices", [b_sh, A, K], mybir.dt.int32,
                           kind="ExternalOutput").ap()
    valid_d = nc.dram_tensor("valid", [b_sh, A, T], mybir.dt.uint8,
                             kind="ExternalOutput").ap()

    with tile.TileContext(nc) as tc:
        with tc.tile_pool(name="const", bufs=1) as cpool, \
             tc.tile_pool(name="xin", bufs=6) as xpool, \
             tc.tile_pool(name="work", bufs=3) as wpool, \
             tc.tile_pool(name="small", bufs=3) as spool, tc.tile_pool(name="vpool", bufs=2) as vpool, \
             tc.tile_pool(name="psA", bufs=2, space="PSUM") as psA, tc.tile_pool(name="psC", bufs=2, space="PSUM") as psC, \
             tc.tile_pool(name="psB", bufs=1, space="PSUM") as psB:

            ident = cpool.tile([P, P], mybir.dt.float32)
            make_identity(nc, ident)

            thr1 = cpool.tile([1, 1], mybir.dt.float32)
            nc.sync.dma_start(thr1[:], thr_d[:])
            thr_b = cpool.tile([P, 1], mybir.dt.float32)
            nc.gpsimd.partition_broadcast(thr_b[:], thr1[:])

            ones_row = cpool.tile([1, P], mybir.dt.float32)
            nc.vector.memset(ones_row[:], 1.0)

            ones16 = cpool.tile([P, K], mybir.dt.uint16)
            nc.vector.memset(ones16[:], 0x3F80)  # bf16 1.0 bit pattern

            thr_neg = cpool.tile([P, 1], mybir.dt.float32)
            nc.vector.tensor_scalar(out=thr_neg[:], in0=thr_b[:],
                                    scalar1=-1.0, scalar2=None,
                                    op0=mybir.AluOpType.mult)

            def stage1(b):
                xts = []
                for h in range(2):
                    xt = xpool.tile([P, T], mybir.dt.float32, tag="x",
                                    name=f"x_{b}_{h}")
                    nc.sync.dma_start(xt[:], x_d[b, h * P:(h + 1) * P, :])
                    xts.append(xt)

                # valid mask out (ACT engine: sign then relu)
                for h in range(2):
                    vs = vpool.tile([P, T], mybir.dt.float32, tag="vsign",
                                    name=f"vs_{b}_{h}")
                    nc.scalar.activation(
                        out=vs[:], in_=xts[h][:],
                        func=mybir.ActivationFunctionType.Sign,
                        bias=thr_neg[:, 0:1], scale=1.0)
                    vt = spool.tile([P, T], mybir.dt.uint8, tag="valid",
                                    name=f"vt_{b}_{h}")
                    nc.scalar.activation(
                        out=vt[:], in_=vs[:],
                        func=mybir.ActivationFunctionType.Relu)
                    nc.sync.dma_start(valid_d[b, h * P:(h + 1) * P, :], vt[:])

                # column max over atoms
                m1 = wpool.tile([P, T], mybir.dt.float32, tag="m1",
                                name=f"m1_{b}")
                nc.vector.tensor_tensor(out=m1[:], in0=xts[0][:],
                                        in1=xts[1][:], op=mybir.AluOpType.max)
                mc = spool.tile([P, T // P], mybir.dt.float32, tag="mc",
                                name=f"mc_{b}")
                for ch in range(4):
                    mt_ps = psA.tile([P, T // 4], mybir.dt.float32,
                                     space="PSUM", tag="mt",
                                     name=f"mt_{b}_{ch}")
                    for c in range(4):
                        nc.tensor.transpose(
                            out=mt_ps[:, c * P:(c + 1) * P],
                            in_=m1[:, (ch * 4 + c) * P:(ch * 4 + c + 1) * P],
                            identity=ident[:])
                    nc.vector.tensor_reduce(
                        out=mc[:, ch * 4:(ch + 1) * 4],
                        in_=mt_ps[:].rearrange("p (c q) -> p c q", q=P),
                        axis=mybir.AxisListType.X, op=mybir.AluOpType.max)
                # clamp with threshold
                nc.vector.tensor_scalar(out=mc[:], in0=mc[:],
                                        scalar1=thr_b[:, 0:1], scalar2=None,
                                        op0=mybir.AluOpType.max)

                # m' row: transpose [128,16] -> [16,128] then flatten
                mt2_ps = psC.tile([T // P, P], mybir.dt.float32, space="PSUM",
                                  tag="mt2", name=f"mt2_{b}")
                nc.tensor.transpose(out=mt2_ps[:], in_=mc[:],
                                    identity=ident[:])
                mts = spool.tile([T // P, P], mybir.dt.float32, tag="mts",
                                 name=f"mts_{b}")
                nc.scalar.activation(
                    out=mts[:], in_=mt2_ps[:],
                    func=mybir.ActivationFunctionType.Copy)
                mrow = spool.tile([1, T], mybir.dt.float32, tag="mrow",
                                  name=f"mrow_{b}")
                nc.sync.dma_start(
                    mrow[:].rearrange("one (c q) -> one c q", q=P), mts[:])

                return xts, mrow

            def stage2(b, xts, mrow):
                y0 = wpool.tile([P, T], mybir.dt.float32, tag="y0",
                                name=f"y0_{b}")
                y1 = wpool.tile([P, T], mybir.dt.float32, tag="y1",
                                name=f"y1_{b}")
                ys = [y0, y1]
                mb_ps = psB.tile([P, T], mybir.dt.float32, space="PSUM",
                                 tag="mb", name=f"mb_{b}")
                for j in range(T // 512):
                    nc.tensor.matmul(
                        out=mb_ps[:, j * 512:(j + 1) * 512],
                        lhsT=ones_row[:],
                        rhs=mrow[0:1, j * 512:(j + 1) * 512],
                        start=True, stop=True)
                for h in range(2):
                    nc.vector.tensor_tensor(
                        out=ys[h][:], in0=xts[h][:], in1=mb_ps[:],
                        op=mybir.AluOpType.is_ge)
                # y = x * (x >= m')
                nc.vector.tensor_tensor(out=ys[0][:], in0=ys[0][:],
                                        in1=xts[0][:],
                                        op=mybir.AluOpType.mult)
                nc.gpsimd.tensor_tensor(out=ys[1][:], in0=ys[1][:],
                                        in1=xts[1][:],
                                        op=mybir.AluOpType.mult)

                i12 = spool.tile([P, 2 * K], mybir.dt.uint16, tag="i12",
                                 name=f"i12_{b}")
                for h in range(2):
                    yh = ys[h]
                    v1t = spool.tile([P, 8], mybir.dt.float32, tag="v1",
                                     name=f"v1_{b}_{h}")
                    v2t = spool.tile([P, 8], mybir.dt.float32, tag="v2",
                                     name=f"v2_{b}_{h}")
                    rep = wpool.tile([P, T], mybir.dt.float32, tag="rep",
                                     name=f"rep_{b}_{h}")
                    nc.vector.max(out=v1t[:], in_=yh[:])
                    nc.vector.max_index(out=i12[:, h * K:h * K + 8],
                                        in_max=v1t[:], in_values=yh[:])
                    nc.vector.match_replace(out=rep[:], in_to_replace=v1t[:],
                                            in_values=yh[:], imm_value=-1.0)
                    nc.vector.max(out=v2t[:], in_=rep[:])
                    nc.vector.max_index(out=i12[:, h * K + 8:(h + 1) * K],
                                        in_max=v2t[:], in_values=rep[:])

                iall = spool.tile([P, 2 * K], mybir.dt.uint32, tag="iall",
                                  name=f"iall_{b}")
                nc.scalar.activation(
                    out=iall[:], in_=i12[:],
                    func=mybir.ActivationFunctionType.Copy)
                for h in range(2):
                    nc.sync.dma_start(
                        idx_d[b, h * P:(h + 1) * P, :],
                        iall[:, h * K:(h + 1) * K].bitcast(mybir.dt.int32))

                # split indices into two 1024-halves for local_scatter
                iallf = spool.tile([P, 2 * K], mybir.dt.float32, tag="iallf",
                                   name=f"iallf_{b}")
                nc.scalar.activation(
                    out=iallf[:], in_=i12[:],
                    func=mybir.ActivationFunctionType.Copy)
                ge = spool.tile([P, 2 * K], mybir.dt.float32, tag="ge",
                                name=f"ge_{b}")
                nc.vector.tensor_scalar(out=ge[:], in0=iallf[:],
                                        scalar1=float(HT),
                                        scalar2=4096.0,
                                        op0=mybir.AluOpType.is_ge,
                                        op1=mybir.AluOpType.mult)
                loi = spool.tile([P, 2 * K], mybir.dt.int16, tag="loi",
                                 name=f"loi_{b}")
                nc.vector.tensor_tensor(out=loi[:], in0=iallf[:], in1=ge[:],
                                        op=mybir.AluOpType.subtract)
                hif = spool.tile([P, 2 * K], mybir.dt.float32, tag="hif",
                                 name=f"hif_{b}")
                nc.vector.tensor_tensor(out=hif[:], in0=iallf[:], in1=ge[:],
                                        op=mybir.AluOpType.add)
                hii = spool.tile([P, 2 * K], mybir.dt.int16, tag="hii",
                                 name=f"hii_{b}")
                nc.vector.tensor_scalar(out=hii[:], in0=hif[:],
                                        scalar1=-(4096.0 + HT), scalar2=None,
                                        op0=mybir.AluOpType.add)
                for h in range(2):
                    for s, idxs in ((0, loi), (1, hii)):
                        ft = spool.tile([P, HT], mybir.dt.uint16, tag="ft",
                                        name=f"ft_{b}_{h}_{s}")
                        nc.gpsimd.local_scatter(
                            out_ap=ft[:], data_ap=ones16[:],
                            idxs_ap=idxs[:, h * K:(h + 1) * K],
                            channels=P, num_elems=HT, num_idxs=K)
                        nc.sync.dma_start(
                            feat_d[b, h * P:(h + 1) * P,
                                   s * HT:(s + 1) * HT], ft[:])

            pending = [stage1(0)]
            if b_sh > 1:
                pending.append(stage1(1))
            for b in range(b_sh):
                if b + 2 < b_sh:
                    pending.append(stage1(b + 2))
                stage2(b, *pending.pop(0))

    nc.compile()
    return nc


def _get_nc():
    if "nc" not in _CACHE:
        _CACHE["nc"] = _build()
    return _CACHE["nc"]


def kernel(x: np.ndarray, threshold: np.ndarray) -> tuple:
    import ml_dtypes
    from concourse.bass_utils import run_bass_kernel_spmd

    nc = _get_nc()
    x = np.ascontiguousarray(np.asarray(x, dtype=np.float32))
    thr = np.asarray(threshold, dtype=np.float32).reshape(1, 1)

    in_maps = [
        {"x": x[c * B_SH:(c + 1) * B_SH], "threshold": thr}
        for c in range(NCORES)
    ]
    import os
    trace = bool(int(os.environ.get("KERNEL_TRACE", "0")))
    res = run_bass_kernel_spmd(nc, in_maps, core_ids=list(range(NCORES)),
                               trace=trace)
    _CACHE["last_result"] = res

    feat = np.empty((B, A, T), dtype=np.float32)
    indices = np.empty((B, A, K), dtype=np.int32)
    valid = np.empty((B, A, T), dtype=bool)
    for c in range(NCORES):
        out = res.results[c]
        sl = slice(c * B_SH, (c + 1) * B_SH)
        feat[sl] = out["feat"].view(ml_dtypes.bfloat16).astype(np.float32)
        indices[sl] = out["indices"]
        valid[sl] = out["valid"].astype(bool)
    return feat, indices, valid
